# revision 1
# baseline (speedup 1.0000x reference)
"""LAHGCN hypergraph-conv kernel for 8 Trainium2 NeuronCores.

Math (per reference):
  smooth(x) = Dv^-1/2 H De^-1 H^T Dv^-1/2 x  (S),  branches k=0..3:
  hidden_k = relu(S(x_k W1_k + 1 b1_k));  out = concat(hidden) W2 + b2;  res = S out.

Strategy: nodes sharded 8-way for matmuls / node-side segment sums, edges
sharded 8-way for edge-side segment sums, AllGather between the two sides.
Segment sums = indirect dma_gather of rows + one-hot matmul on TensorE with
statically-baked (per-input) index/segment streams.  Degree scalings folded
into the y table (dv), the edge pass (de) and post-W2 (dv^2); b1 via rank-1
matmul; b2 via host-side rank-1 s1 = S@1 correction.
"""
import numpy as np

N, E, NNZ = 50000, 20000, 1600000
CONCAT, C_IN, C_HID = 4, 256, 256
C = CONCAT * C_HID            # 1024
C_OUT, C_OUT_P = 40, 64
W = 8
NPC_R, EPC_R = N // W, E // W           # 6250, 2500 real per core
NBLK, EBLK = 49, 20
NPC, EPC = NBLK * 128, EBLK * 128       # 6272, 2560 padded per core
NP_, EP_ = W * NPC, W * EPC             # 50176, 20480
NHALF = NP_ // 2                        # 25088
BATCH = 8                                # gather chunks per dma_gather


def _wrap_idx(idx):
    """[L] int -> [128, L/16] int16 wrapped layout, replicated across q7 cores."""
    L = len(idx)
    assert L % 16 == 0
    a = np.full((16, L // 16), 0, np.int16)
    a[np.arange(L) % 16, np.arange(L) // 16] = idx.astype(np.int16)
    return np.tile(a, (8, 1))


def _streams(rows, segpos, nblk, K):
    """Build flat index stream [nblk*K*128] + seg table [128, nblk*K].
    rows/segpos: list per block of (row_ids, positions 0..127)."""
    L = nblk * K * 128
    idx = np.zeros(L, np.int64)
    seg = np.full((128, nblk * K), -1.0, np.float32)
    for b in range(nblk):
        r, p = rows[b], segpos[b]
        n = len(r)
        assert n <= K * 128
        base = b * K * 128
        idx[base:base + n] = r
        cols = b * K + np.arange(n) // 128
        seg[np.arange(n) % 128, cols] = p.astype(np.float32)
    return idx, seg


def _prep(node_idx, edge_idx, dv_is, de_inv):
    """All host-side index prep. Returns per-core dicts of arrays."""
    nrow = (node_idx // NPC_R) * NPC + node_idx % NPC_R    # node -> y row
    erow = (edge_idx // EPC_R) * EPC + edge_idx % EPC_R    # edge -> ef row
    cores = []
    # dir1: sort by edge
    p1 = np.argsort(edge_idx, kind="stable")
    e1, n1 = edge_idx[p1], nrow[p1]
    # dir2: sort by node
    p2 = np.argsort(node_idx, kind="stable")
    n2, e2 = node_idx[p2], erow[p2]
    per = []
    for c in range(W):
        m1 = (e1 >= c * EPC_R) & (e1 < (c + 1) * EPC_R)
        el = e1[m1] - c * EPC_R
        nr = n1[m1]
        lo_rows, lo_pos, hi_rows, hi_pos = [], [], [], []
        for b in range(EBLK):
            mb = (el >= b * 128) & (el < (b + 1) * 128)
            rb, pb = nr[mb], el[mb] - b * 128
            lo = rb < NHALF
            lo_rows.append(rb[lo]); lo_pos.append(pb[lo])
            hi_rows.append(rb[~lo] - NHALF); hi_pos.append(pb[~lo])
        m2 = (n2 >= c * NPC_R) & (n2 < (c + 1) * NPC_R)
        nl = n2[m2] - c * NPC_R
        er = e2[m2]
        c_rows, c_pos = [], []
        for b in range(NBLK):
            mb = (nl >= b * 128) & (nl < (b + 1) * 128)
            c_rows.append(er[mb]); c_pos.append(nl[mb] - b * 128)
        per.append((lo_rows, lo_pos, hi_rows, hi_pos, c_rows, c_pos))
    KA = max(max((len(r) + 127) // 128 for r in p[0]) for p in per)
    KB = max(max((len(r) + 127) // 128 for r in p[2]) for p in per)
    KC = max(max((len(r) + 127) // 128 for r in p[4]) for p in per)
    KA, KB, KC = max(KA, 1), max(KB, 1), max(KC, 1)
    for c in range(W):
        lo_rows, lo_pos, hi_rows, hi_pos, c_rows, c_pos = per[c]
        iA, sA = _streams(lo_rows, lo_pos, EBLK, KA)
        iB, sB = _streams(hi_rows, hi_pos, EBLK, KB)
        iC, sC = _streams(c_rows, c_pos, NBLK, KC)
        dv = np.zeros(NPC, np.float32)
        dv[:NPC_R] = dv_is[c * NPC_R:(c + 1) * NPC_R]
        de = np.zeros(EPC, np.float32)
        de[:EPC_R] = de_inv[c * EPC_R:(c + 1) * EPC_R]
        cores.append(dict(
            idxA=_wrap_idx(iA), segA=sA, idxB=_wrap_idx(iB), segB=sB,
            idxC=_wrap_idx(iC), segC=sC,
            dv_blk=dv.reshape(NBLK, 128).T.copy(),
            dvsq_blk=(dv * dv).reshape(NBLK, 128).T.copy(),
            de_blk=de.reshape(EBLK, 128).T.copy()))
    return cores, KA, KB, KC


def _build(KA, KB, KC):
    import concourse.bass as bass
    import concourse.mybir as mybir
    from concourse import bacc, masks
    from concourse.tile import TileContext

    f32, i16 = mybir.dt.float32, mybir.dt.int16
    nc = bacc.Bacc("TRN2", num_devices=W)
    T = lambda n, s, d=f32: nc.dram_tensor(n, s, d, kind="ExternalInput")
    xT = T("xT", [CONCAT, C_IN, NPC])
    W1 = T("W1", [CONCAT, C_IN, C_HID])
    b1c = T("b1c", [1, C])
    W2p = T("W2p", [C, C_OUT_P])
    dv_blk = T("dv_blk", [128, NBLK]); dvsq_blk = T("dvsq_blk", [128, NBLK])
    de_blk = T("de_blk", [128, EBLK])
    idxA = T("idxA", [128, EBLK * KA * 8], i16); segA = T("segA", [128, EBLK * KA])
    idxB = T("idxB", [128, EBLK * KB * 8], i16); segB = T("segB", [128, EBLK * KB])
    idxC = T("idxC", [128, NBLK * KC * 8], i16); segC = T("segC", [128, NBLK * KC])
    iota_d = T("iota", [128, 128])
    out_own = nc.dram_tensor("out_own", [NPC, C_OUT_P], f32, kind="ExternalOutput")
    I = lambda n, s: nc.dram_tensor(n, s, f32, kind="Internal")
    S = lambda n, s: nc.dram_tensor(n, s, f32, kind="Internal", addr_space="Shared")
    y_own, y_full = I("y_own", [NPC, C]), S("y_full", [NP_, C])
    ef_own, ef_full = I("ef_own", [EPC, C]), S("ef_full", [EP_, C])
    y2_own, y2_full = I("y2_own", [NPC, C_OUT_P]), S("y2_full", [NP_, C_OUT_P])
    ef2_own, ef2_full = I("ef2_own", [EPC, C_OUT_P]), S("ef2_full", [EP_, C_OUT_P])
    RG = [list(range(W))]
    AG = lambda i, o: nc.gpsimd.collective_compute(
        "AllGather", mybir.AluOpType.bypass, replica_groups=RG, ins=[i[:]], outs=[o[:]])

    with TileContext(nc) as tc:
        with tc.tile_pool(name="const", bufs=1) as cp:
            w1_sb = cp.tile([128, CONCAT * 2 * C_HID], f32)       # [k][q] -> 256 cols each
            for k in range(CONCAT):
                for q in range(2):
                    nc.sync.dma_start(
                        w1_sb[:, (k * 2 + q) * C_HID:(k * 2 + q + 1) * C_HID],
                        W1[k, q * 128:(q + 1) * 128, :])
            w2_sb = cp.tile([128, 8 * C_OUT_P], f32)
            for f in range(8):
                nc.sync.dma_start(w2_sb[:, f * C_OUT_P:(f + 1) * C_OUT_P],
                                  W2p[f * 128:(f + 1) * 128, :])
            b1_sb = cp.tile([1, C], f32); nc.sync.dma_start(b1_sb[:], b1c[:])
            ones_sb = cp.tile([1, 128], f32); nc.vector.memset(ones_sb[:], 1.0)
            iota_sb = cp.tile([128, 128], f32); nc.sync.dma_start(iota_sb[:], iota_d[:])
            ident = cp.tile([128, 128], f32); masks.make_identity(nc, ident[:])
            dv_sb = cp.tile([128, NBLK], f32); nc.sync.dma_start(dv_sb[:], dv_blk[:])
            dvsq_sb = cp.tile([128, NBLK], f32); nc.sync.dma_start(dvsq_sb[:], dvsq_blk[:])
            de_sb = cp.tile([128, EBLK], f32); nc.sync.dma_start(de_sb[:], de_blk[:])
            iA = cp.tile([128, EBLK * KA * 8], i16); nc.sync.dma_start(iA[:], idxA[:])
            iB = cp.tile([128, EBLK * KB * 8], i16); nc.sync.dma_start(iB[:], idxB[:])
            iC = cp.tile([128, NBLK * KC * 8], i16); nc.sync.dma_start(iC[:], idxC[:])
            sA = cp.tile([128, EBLK * KA], f32); nc.sync.dma_start(sA[:], segA[:])
            sB = cp.tile([128, EBLK * KB], f32); nc.sync.dma_start(sB[:], segB[:])
            sC = cp.tile([128, NBLK * KC], f32); nc.sync.dma_start(sC[:], segC[:])

            mm = lambda *a, **kw: nc.tensor.matmul(*a, skip_group_check=True, **kw)

            def seg_pass(nblk, K, idx_sb, seg_sb, src_ap, elem, pool, psum_ap_of_blk,
                         start_stream, stop_stream):
                """Gather+one-hot-matmul accumulation over one stream."""
                nbat = (K + BATCH - 1) // BATCH
                for b in range(nblk):
                    for s in range(nbat):
                        k0 = b * K + s * BATCH
                        nch = min(BATCH, K - s * BATCH)
                        g = pool.tile([128, BATCH, elem], f32, tag="gat%d" % elem)
                        nc.gpsimd.dma_gather(
                            out_ap=g[:, :nch, :], in_ap=src_ap,
                            idxs_ap=idx_sb[:, k0 * 8:(k0 + nch) * 8],
                            num_idxs=nch * 128, num_idxs_reg=nch * 128,
                            elem_size=elem)
                        oh = pool.tile([128, BATCH, 128], f32, tag="oh")
                        nc.vector.tensor_tensor(
                            out=oh[:, :nch, :],
                            in0=iota_sb[:, None, :].broadcast_to([128, nch, 128]),
                            in1=seg_sb[:, k0:k0 + nch, None].broadcast_to([128, nch, 128]),
                            op=mybir.AluOpType.is_equal)
                        ps = psum_ap_of_blk(b)
                        for j in range(nch):
                            first = start_stream and (s == 0 and j == 0)
                            last = stop_stream and (k0 + j == b * K + K - 1)
                            for h in range((elem + 511) // 512):
                                w_ = min(512, elem - h * 512)
                                mm(ps[:, h * 512:h * 512 + w_],
                                   lhsT=oh[:, j, :], rhs=g[:, j, h * 512:h * 512 + w_],
                                   start=first, stop=last)

            # ---- phase A: y = dv * (x @ W1 + 1 b1) ----
            with tc.tile_pool(name="pa", bufs=3) as pa, \
                 tc.tile_pool(name="pap", bufs=2, space="PSUM") as pap:
                for b in range(NBLK):
                    ps = pap.tile([128, C], f32, tag="psA")
                    mm(ps[:, :512], lhsT=ones_sb[:, :], rhs=b1_sb[:, :512], start=True, stop=False)
                    mm(ps[:, 512:], lhsT=ones_sb[:, :], rhs=b1_sb[:, 512:], start=True, stop=False)
                    for k in range(CONCAT):
                        for q in range(2):
                            xt = pa.tile([128, 128], f32, tag="xt")
                            nc.sync.dma_start(xt[:], xT[k, q * 128:(q + 1) * 128,
                                                        b * 128:(b + 1) * 128])
                            mm(ps[:, k * C_HID:(k + 1) * C_HID], lhsT=xt[:],
                               rhs=w1_sb[:, (k * 2 + q) * C_HID:(k * 2 + q + 1) * C_HID],
                               start=False, stop=(q == 1))
                    y_sb = pa.tile([128, C], f32, tag="ysb")
                    nc.vector.tensor_tensor(
                        out=y_sb[:], in0=ps[:],
                        in1=dv_sb[:, b:b + 1].broadcast_to([128, C]),
                        op=mybir.AluOpType.mult)
                    nc.sync.dma_start(y_own[b * 128:(b + 1) * 128, :], y_sb[:])
            AG(y_own, y_full)

            # ---- phase B: ef = de * (H^T y) over own edges ----
            with tc.tile_pool(name="pb", bufs=2) as pb, \
                 tc.tile_pool(name="pbp", bufs=2, space="PSUM") as pbp:
                pstiles = {}
                def psB(b):
                    if b not in pstiles:
                        pstiles[b] = pbp.tile([128, C], f32, tag="psB", name="psB%d" % b)
                    return pstiles[b]
                for b in range(EBLK):
                    seg_pass(1, KA, iA[:, b * KA * 8:], sA[:, b * KA:], y_full[0:NHALF, :],
                             C, pb, lambda _b: psB(b), True, False)
                    seg_pass(1, KB, iB[:, b * KB * 8:], sB[:, b * KB:],
                             y_full[NHALF:NP_, :], C, pb, lambda _b: psB(b), False, True)
                    ef_sb = pb.tile([128, C], f32, tag="efsb")
                    nc.vector.tensor_tensor(
                        out=ef_sb[:], in0=psB(b)[:],
                        in1=de_sb[:, b:b + 1].broadcast_to([128, C]),
                        op=mybir.AluOpType.mult)
                    nc.sync.dma_start(ef_own[b * 128:(b + 1) * 128, :], ef_sb[:])
                    del pstiles[b]
            AG(ef_own, ef_full)

            # ---- phase C: u = relu(H ef); y2 = dv^2 * (u @ W2) ----
            with tc.tile_pool(name="pc", bufs=2) as pc, \
                 tc.tile_pool(name="pcp", bufs=2, space="PSUM") as pcp, \
                 tc.tile_pool(name="pct", bufs=1, space="PSUM") as pct:
                for b in range(NBLK):
                    pz = pcp.tile([128, C], f32, tag="psC")
                    seg_pass(1, KC, iC[:, b * KC * 8:], sC[:, b * KC:], ef_full[:],
                             C, pc, lambda _b: pz, True, True)
                    u_sb = pc.tile([128, C], f32, tag="usb")
                    nc.scalar.activation(out=u_sb[:], in_=pz[:],
                                         func=mybir.ActivationFunctionType.Relu)
                    pt = pct.tile([128, C], f32, tag="ptC")
                    for f in range(8):
                        nc.tensor.transpose(pt[:, f * 128:(f + 1) * 128],
                                            u_sb[:, f * 128:(f + 1) * 128], ident[:])
                    ut_sb = pc.tile([128, C], f32, tag="utsb")
                    nc.vector.tensor_copy(ut_sb[:], pt[:])
                    po = pct.tile([128, C_OUT_P], f32, tag="poC")
                    for f in range(8):
                        mm(po[:], lhsT=ut_sb[:, f * 128:(f + 1) * 128],
                           rhs=w2_sb[:, f * C_OUT_P:(f + 1) * C_OUT_P],
                           start=(f == 0), stop=(f == 7))
                    y2_sb = pc.tile([128, C_OUT_P], f32, tag="y2sb")
                    nc.vector.tensor_tensor(
                        out=y2_sb[:], in0=po[:],
                        in1=dvsq_sb[:, b:b + 1].broadcast_to([128, C_OUT_P]),
                        op=mybir.AluOpType.mult)
                    nc.sync.dma_start(y2_own[b * 128:(b + 1) * 128, :], y2_sb[:])
            AG(y2_own, y2_full)

            # ---- phase D: ef2 = de * (H^T y2) ----
            with tc.tile_pool(name="pd", bufs=2) as pd, \
                 tc.tile_pool(name="pdp", bufs=2, space="PSUM") as pdp:
                for b in range(EBLK):
                    ps2 = pdp.tile([128, C_OUT_P], f32, tag="psD")
                    seg_pass(1, KA, iA[:, b * KA * 8:], sA[:, b * KA:],
                             y2_full[0:NHALF, :], C_OUT_P, pd, lambda _b: ps2, True, False)
                    seg_pass(1, KB, iB[:, b * KB * 8:], sB[:, b * KB:],
                             y2_full[NHALF:NP_, :], C_OUT_P, pd, lambda _b: ps2,
                             False, True)
                    e2_sb = pd.tile([128, C_OUT_P], f32, tag="e2sb")
                    nc.vector.tensor_tensor(
                        out=e2_sb[:], in0=ps2[:],
                        in1=de_sb[:, b:b + 1].broadcast_to([128, C_OUT_P]),
                        op=mybir.AluOpType.mult)
                    nc.sync.dma_start(ef2_own[b * 128:(b + 1) * 128, :], e2_sb[:])
            AG(ef2_own, ef2_full)

            # ---- phase E: res = dv * (H ef2) ----
            with tc.tile_pool(name="pe", bufs=2) as pe_, \
                 tc.tile_pool(name="pep", bufs=2, space="PSUM") as pep:
                for b in range(NBLK):
                    pz2 = pep.tile([128, C_OUT_P], f32, tag="psE")
                    seg_pass(1, KC, iC[:, b * KC * 8:], sC[:, b * KC:], ef2_full[:],
                             C_OUT_P, pe_, lambda _b: pz2, True, True)
                    o_sb = pe_.tile([128, C_OUT_P], f32, tag="osb")
                    nc.vector.tensor_tensor(
                        out=o_sb[:], in0=pz2[:],
                        in1=dv_sb[:, b:b + 1].broadcast_to([128, C_OUT_P]),
                        op=mybir.AluOpType.mult)
                    nc.sync.dma_start(out_own[b * 128:(b + 1) * 128, :], o_sb[:])
    nc.finalize()
    return nc


_CACHE = {}


def kernel(x_list, W1, b1, W2, b2, node_idx, edge_idx, n_edges, _trace=False):
    from concourse import bass_utils
    x_list = np.asarray(x_list, np.float32); W1 = np.asarray(W1, np.float32)
    b1 = np.asarray(b1, np.float32); W2 = np.asarray(W2, np.float32)
    b2 = np.asarray(b2, np.float32)
    node_idx = np.asarray(node_idx, np.int32); edge_idx = np.asarray(edge_idx, np.int32)

    dv = np.bincount(node_idx, minlength=N).astype(np.float32)
    de = np.bincount(edge_idx, minlength=E).astype(np.float32)
    dv_is = np.where(dv > 0, 1.0 / np.sqrt(np.maximum(dv, 1.0)), 0.0).astype(np.float32)
    de_inv = np.where(de > 0, 1.0 / np.maximum(de, 1.0), 0.0).astype(np.float32)
    # s1 = S @ 1 for the host-side b2 rank-1 term
    ef_t = np.bincount(edge_idx, weights=dv_is[node_idx], minlength=E) * de_inv
    s1 = dv_is * np.bincount(node_idx, weights=ef_t[edge_idx], minlength=N)

    cores, KA, KB, KC = _prep(node_idx, edge_idx, dv_is, de_inv)
    key = (KA, KB, KC)
    if key not in _CACHE:
        _CACHE[key] = _build(KA, KB, KC)
    nc = _CACHE[key]

    W2p = np.zeros((C, C_OUT_P), np.float32)
    W2p[:, :C_OUT] = W2
    iota_np = np.tile(np.arange(128, dtype=np.float32), (128, 1))
    in_maps = []
    for c in range(W):
        xTc = np.zeros((CONCAT, C_IN, NPC), np.float32)
        xTc[:, :, :NPC_R] = x_list[:, c * NPC_R:(c + 1) * NPC_R, :].transpose(0, 2, 1)
        m = dict(xT=xTc, W1=W1, b1c=b1.reshape(1, C).copy(), W2p=W2p, iota=iota_np,
                 **cores[c])
        in_maps.append(m)
    try:
        res = bass_utils.run_bass_kernel_spmd(nc, in_maps, core_ids=list(range(W)),
                                              trace=_trace)
    except ModuleNotFoundError:
        res = bass_utils.run_bass_kernel_spmd(nc, in_maps, core_ids=list(range(W)),
                                              trace=False)
    out = np.empty((N, C_OUT), np.float32)
    for c in range(W):
        out[c * NPC_R:(c + 1) * NPC_R] = res.results[c]["out_own"][:NPC_R, :C_OUT]
    out += np.outer(s1, b2)
    kernel._last = res
    return out



# revision 3
# speedup vs baseline: 1.3871x; 1.3871x over previous
"""LAHGCN hypergraph-conv kernel for 8 Trainium2 NeuronCores.

Math (per reference):
  smooth(x) = Dv^-1/2 H De^-1 H^T Dv^-1/2 x  (S),  branches k=0..3:
  hidden_k = relu(S(x_k W1_k + 1 b1_k));  out = concat(hidden) W2 + b2;  res = S out.

Strategy: nodes sharded 8-way for matmuls / node-side segment sums, edges
sharded 8-way for edge-side segment sums, AllGather between the two sides.
Segment sums = indirect dma_gather of rows + one-hot matmul on TensorE with
statically-baked (per-input) index/segment streams.  All gather tables and
one-hot operands are bf16 (fp32 PSUM accumulate) - 4x TensorE throughput,
FWL weight loads and half the gather DMA bytes vs fp32.  Degree scalings
folded into the y table (dv), the edge pass (de) and post-W2 (dv^2); b1 via
rank-1 matmul; b2 via host-side rank-1 s1 = S@1 correction.
"""
import numpy as np

N, E, NNZ = 50000, 20000, 1600000
CONCAT, C_IN, C_HID = 4, 256, 256
C = CONCAT * C_HID            # 1024
C_OUT, C_OUT_P = 40, 64
C2P = 128                     # padded col width of the 2nd-smooth tables (bf16, 256B rows)
W = 8
NPC_R, EPC_R = N // W, E // W           # 6250, 2500 real per core
NBLK, EBLK = 49, 20
NPC, EPC = NBLK * 128, EBLK * 128       # 6272, 2560 padded per core
NP_, EP_ = W * NPC, W * EPC             # 50176, 20480
NHALF = NP_ // 2                        # 25088
BATCH = 8                                # gather chunks per dma_gather


def _bf16(a):
    import ml_dtypes
    return np.asarray(a).astype(ml_dtypes.bfloat16)


def _wrap_idx(idx):
    """[L] int -> [128, L/16] int16 wrapped layout, replicated across q7 cores."""
    L = len(idx)
    assert L % 16 == 0
    a = np.full((16, L // 16), 0, np.int16)
    a[np.arange(L) % 16, np.arange(L) // 16] = idx.astype(np.int16)
    return np.tile(a, (8, 1))


def _streams(rows, segpos, nblk, K):
    """Build flat index stream [nblk*K*128] + seg table [128, nblk*K].
    rows/segpos: list per block of (row_ids, positions 0..127)."""
    L = nblk * K * 128
    idx = np.zeros(L, np.int64)
    seg = np.full((128, nblk * K), -1.0, np.float32)
    for b in range(nblk):
        r, p = rows[b], segpos[b]
        n = len(r)
        assert n <= K * 128
        base = b * K * 128
        idx[base:base + n] = r
        cols = b * K + np.arange(n) // 128
        seg[np.arange(n) % 128, cols] = p.astype(np.float32)
    return idx, seg


def _prep(node_idx, edge_idx, dv_is, de_inv):
    """All host-side index prep. Returns per-core dicts of arrays."""
    nrow = (node_idx // NPC_R) * NPC + node_idx % NPC_R    # node -> y row
    erow = (edge_idx // EPC_R) * EPC + edge_idx % EPC_R    # edge -> ef row
    # dir1: sort by edge
    p1 = np.argsort(edge_idx, kind="stable")
    e1, n1 = edge_idx[p1], nrow[p1]
    # dir2: sort by node
    p2 = np.argsort(node_idx, kind="stable")
    n2, e2 = node_idx[p2], erow[p2]
    per = []
    for c in range(W):
        m1 = (e1 >= c * EPC_R) & (e1 < (c + 1) * EPC_R)
        el = e1[m1] - c * EPC_R
        nr = n1[m1]
        lo_rows, lo_pos, hi_rows, hi_pos = [], [], [], []
        for b in range(EBLK):
            mb = (el >= b * 128) & (el < (b + 1) * 128)
            rb, pb = nr[mb], el[mb] - b * 128
            lo = rb < NHALF
            lo_rows.append(rb[lo]); lo_pos.append(pb[lo])
            hi_rows.append(rb[~lo] - NHALF); hi_pos.append(pb[~lo])
        m2 = (n2 >= c * NPC_R) & (n2 < (c + 1) * NPC_R)
        nl = n2[m2] - c * NPC_R
        er = e2[m2]
        c_rows, c_pos = [], []
        for b in range(NBLK):
            mb = (nl >= b * 128) & (nl < (b + 1) * 128)
            c_rows.append(er[mb]); c_pos.append(nl[mb] - b * 128)
        per.append((lo_rows, lo_pos, hi_rows, hi_pos, c_rows, c_pos))
    KA = max(max((len(r) + 127) // 128 for r in p[0]) for p in per)
    KB = max(max((len(r) + 127) // 128 for r in p[2]) for p in per)
    KC = max(max((len(r) + 127) // 128 for r in p[4]) for p in per)
    KA, KB, KC = max(KA, 1), max(KB, 1), max(KC, 1)
    cores = []
    for c in range(W):
        lo_rows, lo_pos, hi_rows, hi_pos, c_rows, c_pos = per[c]
        iA, sA = _streams(lo_rows, lo_pos, EBLK, KA)
        iB, sB = _streams(hi_rows, hi_pos, EBLK, KB)
        iC, sC = _streams(c_rows, c_pos, NBLK, KC)
        dv = np.zeros(NPC, np.float32)
        dv[:NPC_R] = dv_is[c * NPC_R:(c + 1) * NPC_R]
        de = np.zeros(EPC, np.float32)
        de[:EPC_R] = de_inv[c * EPC_R:(c + 1) * EPC_R]
        cores.append(dict(
            idxA=_wrap_idx(iA), segA=_bf16(sA), idxB=_wrap_idx(iB), segB=_bf16(sB),
            idxC=_wrap_idx(iC), segC=_bf16(sC),
            dv_blk=dv.reshape(NBLK, 128).T.copy(),
            dvsq_blk=(dv * dv).reshape(NBLK, 128).T.copy(),
            de_blk=de.reshape(EBLK, 128).T.copy()))
    return cores, KA, KB, KC


def _build(KA, KB, KC):
    import concourse.bass as bass
    import concourse.mybir as mybir
    from concourse import bacc, masks
    from concourse.tile import TileContext

    f32, bf16, i16 = mybir.dt.float32, mybir.dt.bfloat16, mybir.dt.int16
    nc = bacc.Bacc("TRN2", num_devices=W)
    T = lambda n, s, d=f32: nc.dram_tensor(n, s, d, kind="ExternalInput")
    xT = T("xT", [CONCAT, C_IN, NPC])
    W1 = T("W1", [CONCAT, C_IN, C_HID])
    b1c = T("b1c", [1, C])
    W2p = T("W2p", [C, C_OUT_P], bf16)
    dv_blk = T("dv_blk", [128, NBLK]); dvsq_blk = T("dvsq_blk", [128, NBLK])
    de_blk = T("de_blk", [128, EBLK])
    idxA = T("idxA", [128, EBLK * KA * 8], i16); segA = T("segA", [128, EBLK * KA], bf16)
    idxB = T("idxB", [128, EBLK * KB * 8], i16); segB = T("segB", [128, EBLK * KB], bf16)
    idxC = T("idxC", [128, NBLK * KC * 8], i16); segC = T("segC", [128, NBLK * KC], bf16)
    iota_d = T("iota", [128, 128], bf16)
    out_own = nc.dram_tensor("out_own", [NPC, C_OUT_P], f32, kind="ExternalOutput")
    I = lambda n, s, d: nc.dram_tensor(n, s, d, kind="Internal")
    S = lambda n, s, d: nc.dram_tensor(n, s, d, kind="Internal", addr_space="Shared")
    y_own, y_full = I("y_own", [NPC, C], bf16), S("y_full", [NP_, C], bf16)
    ef_own, ef_full = I("ef_own", [EPC, C], bf16), S("ef_full", [EP_, C], bf16)
    y2_own, y2_full = I("y2_own", [NPC, C2P], bf16), S("y2_full", [NP_, C2P], bf16)
    ef2_own, ef2_full = I("ef2_own", [EPC, C2P], bf16), S("ef2_full", [EP_, C2P], bf16)
    RG = [list(range(W))]
    AG = lambda i, o: nc.gpsimd.collective_compute(
        "AllGather", mybir.AluOpType.bypass, replica_groups=RG, ins=[i[:]], outs=[o[:]])

    with TileContext(nc) as tc:
        with tc.tile_pool(name="const", bufs=1) as cp:
            w1_sb = cp.tile([128, CONCAT * 2 * C_HID], f32)       # [k][q] -> 256 cols each
            for k in range(CONCAT):
                for q in range(2):
                    nc.sync.dma_start(
                        w1_sb[:, (k * 2 + q) * C_HID:(k * 2 + q + 1) * C_HID],
                        W1[k, q * 128:(q + 1) * 128, :])
            w2_sb = cp.tile([128, 8 * C_OUT_P], bf16)
            for f in range(8):
                nc.sync.dma_start(w2_sb[:, f * C_OUT_P:(f + 1) * C_OUT_P],
                                  W2p[f * 128:(f + 1) * 128, :])
            b1_sb = cp.tile([1, C], f32); nc.sync.dma_start(b1_sb[:], b1c[:])
            ones_sb = cp.tile([1, 128], f32); nc.vector.memset(ones_sb[:], 1.0)
            iota_sb = cp.tile([128, 128], bf16); nc.sync.dma_start(iota_sb[:], iota_d[:])
            ident = cp.tile([128, 128], bf16); masks.make_identity(nc, ident[:])
            dv_sb = cp.tile([128, NBLK], f32); nc.sync.dma_start(dv_sb[:], dv_blk[:])
            dvsq_sb = cp.tile([128, NBLK], f32); nc.sync.dma_start(dvsq_sb[:], dvsq_blk[:])
            de_sb = cp.tile([128, EBLK], f32); nc.sync.dma_start(de_sb[:], de_blk[:])
            iA = cp.tile([128, EBLK * KA * 8], i16); nc.sync.dma_start(iA[:], idxA[:])
            iB = cp.tile([128, EBLK * KB * 8], i16); nc.sync.dma_start(iB[:], idxB[:])
            iC = cp.tile([128, NBLK * KC * 8], i16); nc.sync.dma_start(iC[:], idxC[:])
            sA = cp.tile([128, EBLK * KA], bf16); nc.sync.dma_start(sA[:], segA[:])
            sB = cp.tile([128, EBLK * KB], bf16); nc.sync.dma_start(sB[:], segB[:])
            sC = cp.tile([128, NBLK * KC], bf16); nc.sync.dma_start(sC[:], segC[:])

            mm = lambda *a, **kw: nc.tensor.matmul(*a, skip_group_check=True, **kw)

            def seg_pass(K, idx_sb, seg_sb, src_ap, elem, pool, psum_ap,
                         start_stream, stop_stream):
                """Gather+one-hot-matmul accumulation over one block's stream."""
                nbat = (K + BATCH - 1) // BATCH
                for s in range(nbat):
                    k0 = s * BATCH
                    nch = min(BATCH, K - s * BATCH)
                    g = pool.tile([128, BATCH, elem], bf16, tag="gat%d" % elem)
                    nc.gpsimd.dma_gather(
                        out_ap=g[:, :nch, :], in_ap=src_ap,
                        idxs_ap=idx_sb[:, k0 * 8:(k0 + nch) * 8],
                        num_idxs=nch * 128, num_idxs_reg=nch * 128,
                        elem_size=elem)
                    oh = pool.tile([128, BATCH, 128], bf16, tag="oh")
                    nc.vector.tensor_tensor(
                        out=oh[:, :nch, :],
                        in0=iota_sb[:, None, :].broadcast_to([128, nch, 128]),
                        in1=seg_sb[:, k0:k0 + nch, None].broadcast_to([128, nch, 128]),
                        op=mybir.AluOpType.is_equal)
                    for j in range(nch):
                        first = start_stream and (s == 0 and j == 0)
                        last = stop_stream and (k0 + j == K - 1)
                        for h in range((elem + 511) // 512):
                            w_ = min(512, elem - h * 512)
                            mm(psum_ap[:, h * 512:h * 512 + w_],
                               lhsT=oh[:, j, :], rhs=g[:, j, h * 512:h * 512 + w_],
                               start=first, stop=last)

            # ---- phase A: y = dv * (x @ W1 + 1 b1) ----
            with nc.named_scope("phA"), \
                 tc.tile_pool(name="pa", bufs=3) as pa, \
                 tc.tile_pool(name="pap", bufs=2, space="PSUM") as pap:
                for b in range(NBLK):
                    ps = pap.tile([128, C], f32, tag="psA")
                    mm(ps[:, :512], lhsT=ones_sb[:, :], rhs=b1_sb[:, :512], start=True, stop=False)
                    mm(ps[:, 512:], lhsT=ones_sb[:, :], rhs=b1_sb[:, 512:], start=True, stop=False)
                    for k in range(CONCAT):
                        for q in range(2):
                            xt = pa.tile([128, 128], f32, tag="xt")
                            nc.sync.dma_start(xt[:], xT[k, q * 128:(q + 1) * 128,
                                                        b * 128:(b + 1) * 128])
                            mm(ps[:, k * C_HID:(k + 1) * C_HID], lhsT=xt[:],
                               rhs=w1_sb[:, (k * 2 + q) * C_HID:(k * 2 + q + 1) * C_HID],
                               start=False, stop=(q == 1))
                    y_sb = pa.tile([128, C], bf16, tag="ysb")
                    nc.vector.tensor_tensor(
                        out=y_sb[:], in0=ps[:],
                        in1=dv_sb[:, b:b + 1].broadcast_to([128, C]),
                        op=mybir.AluOpType.mult)
                    nc.sync.dma_start(y_own[b * 128:(b + 1) * 128, :], y_sb[:])
            with nc.named_scope("AGy"):
                AG(y_own, y_full)

            # ---- phase B: ef = de * (H^T y) over own edges ----
            with nc.named_scope("phB"), \
                 tc.tile_pool(name="pb", bufs=2) as pb, \
                 tc.tile_pool(name="pbp", bufs=2, space="PSUM") as pbp:
                for b in range(EBLK):
                    psB = pbp.tile([128, C], f32, tag="psB")
                    seg_pass(KA, iA[:, b * KA * 8:], sA[:, b * KA:], y_full[0:NHALF, :],
                             C, pb, psB, True, False)
                    seg_pass(KB, iB[:, b * KB * 8:], sB[:, b * KB:],
                             y_full[NHALF:NP_, :], C, pb, psB, False, True)
                    ef_sb = pb.tile([128, C], bf16, tag="efsb")
                    nc.vector.tensor_tensor(
                        out=ef_sb[:], in0=psB[:],
                        in1=de_sb[:, b:b + 1].broadcast_to([128, C]),
                        op=mybir.AluOpType.mult)
                    nc.sync.dma_start(ef_own[b * 128:(b + 1) * 128, :], ef_sb[:])
            with nc.named_scope("AGef"):
                AG(ef_own, ef_full)

            # ---- phase C: u = relu(H ef); y2 = dv^2 * (u @ W2) ----
            with nc.named_scope("phC"), \
                 tc.tile_pool(name="pc", bufs=2) as pc, \
                 tc.tile_pool(name="pcp", bufs=2, space="PSUM") as pcp, \
                 tc.tile_pool(name="pct", bufs=1, space="PSUM") as pct:
                for b in range(NBLK):
                    pz = pcp.tile([128, C], f32, tag="psC")
                    seg_pass(KC, iC[:, b * KC * 8:], sC[:, b * KC:], ef_full[:],
                             C, pc, pz, True, True)
                    u_sb = pc.tile([128, C], bf16, tag="usb")
                    nc.scalar.activation(out=u_sb[:], in_=pz[:],
                                         func=mybir.ActivationFunctionType.Relu)
                    pt = pct.tile([128, C], bf16, tag="ptC")
                    for f in range(8):
                        nc.tensor.transpose(pt[:, f * 128:(f + 1) * 128],
                                            u_sb[:, f * 128:(f + 1) * 128], ident[:])
                    ut_sb = pc.tile([128, C], bf16, tag="utsb")
                    nc.vector.tensor_copy(ut_sb[:], pt[:])
                    po = pct.tile([128, C_OUT_P], f32, tag="poC")
                    for f in range(8):
                        mm(po[:], lhsT=ut_sb[:, f * 128:(f + 1) * 128],
                           rhs=w2_sb[:, f * C_OUT_P:(f + 1) * C_OUT_P],
                           start=(f == 0), stop=(f == 7))
                    y2_sb = pc.tile([128, C2P], bf16, tag="y2sb")
                    nc.vector.memset(y2_sb[:, C_OUT_P:], 0.0)
                    nc.vector.tensor_tensor(
                        out=y2_sb[:, :C_OUT_P], in0=po[:],
                        in1=dvsq_sb[:, b:b + 1].broadcast_to([128, C_OUT_P]),
                        op=mybir.AluOpType.mult)
                    nc.sync.dma_start(y2_own[b * 128:(b + 1) * 128, :], y2_sb[:])
            with nc.named_scope("AGy2"):
                AG(y2_own, y2_full)

            # ---- phase D: ef2 = de * (H^T y2) ----
            with nc.named_scope("phD"), \
                 tc.tile_pool(name="pd", bufs=2) as pd, \
                 tc.tile_pool(name="pdp", bufs=2, space="PSUM") as pdp:
                for b in range(EBLK):
                    ps2 = pdp.tile([128, C2P], f32, tag="psD")
                    seg_pass(KA, iA[:, b * KA * 8:], sA[:, b * KA:],
                             y2_full[0:NHALF, :], C2P, pd, ps2, True, False)
                    seg_pass(KB, iB[:, b * KB * 8:], sB[:, b * KB:],
                             y2_full[NHALF:NP_, :], C2P, pd, ps2, False, True)
                    e2_sb = pd.tile([128, C2P], bf16, tag="e2sb")
                    nc.vector.tensor_tensor(
                        out=e2_sb[:], in0=ps2[:],
                        in1=de_sb[:, b:b + 1].broadcast_to([128, C2P]),
                        op=mybir.AluOpType.mult)
                    nc.sync.dma_start(ef2_own[b * 128:(b + 1) * 128, :], e2_sb[:])
            with nc.named_scope("AGef2"):
                AG(ef2_own, ef2_full)

            # ---- phase E: res = dv * (H ef2) ----
            with nc.named_scope("phE"), \
                 tc.tile_pool(name="pe", bufs=2) as pe_, \
                 tc.tile_pool(name="pep", bufs=2, space="PSUM") as pep:
                for b in range(NBLK):
                    pz2 = pep.tile([128, C2P], f32, tag="psE")
                    seg_pass(KC, iC[:, b * KC * 8:], sC[:, b * KC:], ef2_full[:],
                             C2P, pe_, pz2, True, True)
                    o_sb = pe_.tile([128, C_OUT_P], f32, tag="osb")
                    nc.vector.tensor_tensor(
                        out=o_sb[:], in0=pz2[:, :C_OUT_P],
                        in1=dv_sb[:, b:b + 1].broadcast_to([128, C_OUT_P]),
                        op=mybir.AluOpType.mult)
                    nc.sync.dma_start(out_own[b * 128:(b + 1) * 128, :], o_sb[:])
    nc.finalize()
    return nc


_CACHE = {}


def kernel(x_list, W1, b1, W2, b2, node_idx, edge_idx, n_edges, _trace=False):
    from concourse import bass_utils
    x_list = np.asarray(x_list, np.float32); W1 = np.asarray(W1, np.float32)
    b1 = np.asarray(b1, np.float32); W2 = np.asarray(W2, np.float32)
    b2 = np.asarray(b2, np.float32)
    node_idx = np.asarray(node_idx, np.int32); edge_idx = np.asarray(edge_idx, np.int32)

    dv = np.bincount(node_idx, minlength=N).astype(np.float32)
    de = np.bincount(edge_idx, minlength=E).astype(np.float32)
    dv_is = np.where(dv > 0, 1.0 / np.sqrt(np.maximum(dv, 1.0)), 0.0).astype(np.float32)
    de_inv = np.where(de > 0, 1.0 / np.maximum(de, 1.0), 0.0).astype(np.float32)
    # s1 = S @ 1 for the host-side b2 rank-1 term
    ef_t = np.bincount(edge_idx, weights=dv_is[node_idx], minlength=E) * de_inv
    s1 = dv_is * np.bincount(node_idx, weights=ef_t[edge_idx], minlength=N)

    cores, KA, KB, KC = _prep(node_idx, edge_idx, dv_is, de_inv)
    key = (KA, KB, KC)
    if key not in _CACHE:
        _CACHE[key] = _build(KA, KB, KC)
    nc = _CACHE[key]

    W2p = np.zeros((C, C_OUT_P), np.float32)
    W2p[:, :C_OUT] = W2
    iota_np = np.tile(np.arange(128, dtype=np.float32), (128, 1))
    in_maps = []
    for c in range(W):
        xTc = np.zeros((CONCAT, C_IN, NPC), np.float32)
        xTc[:, :, :NPC_R] = x_list[:, c * NPC_R:(c + 1) * NPC_R, :].transpose(0, 2, 1)
        m = dict(xT=xTc, W1=W1, b1c=b1.reshape(1, C).copy(), W2p=_bf16(W2p),
                 iota=_bf16(iota_np), **cores[c])
        in_maps.append(m)
    try:
        res = bass_utils.run_bass_kernel_spmd(nc, in_maps, core_ids=list(range(W)),
                                              trace=_trace)
    except ModuleNotFoundError:
        res = bass_utils.run_bass_kernel_spmd(nc, in_maps, core_ids=list(range(W)),
                                              trace=False)
    out = np.empty((N, C_OUT), np.float32)
    for c in range(W):
        out[c * NPC_R:(c + 1) * NPC_R] = res.results[c]["out_own"][:NPC_R, :C_OUT]
    out += np.outer(s1, b2)
    kernel._last = res
    return out


# revision 5
# speedup vs baseline: 2.0797x; 1.4994x over previous
"""LAHGCN hypergraph-conv kernel for 8 Trainium2 NeuronCores.

Math (per reference):
  smooth(x) = Dv^-1/2 H De^-1 H^T Dv^-1/2 x  (S),  branches k=0..3:
  hidden_k = relu(S(x_k W1_k + 1 b1_k));  out = concat(hidden) W2 + b2;  res = S out.

Strategy:
  * First smooth (1024 features): nodes sharded 8-way for the x@W1 matmuls and
    node-side segment sums, edges sharded 8-way for edge-side segment sums,
    cheap intra-chip AllGathers between sides.  Segment sums = indirect
    dma_gather of 2KB bf16 rows + one-hot matmul on TensorE (fp32 PSUM) with
    statically-baked per-input index/segment streams.
  * Second smooth (64 features): the per-row gather is descriptor-bound on the
    GpSimd Q7 (8.5ns/row), so instead stream a host-baked dense block matrix
    P = Dv^-1/2 H De^-1/2 (bf16, one 128x128 block per node/edge block pair)
    through TensorE: Z = P^T y2, res = P Z.  DMA/TensorE-bound, zero
    descriptors.
  Degree scalings folded into the y table (dv), the edge pass (de), post-W2
  (dv) and the P entries; b1 via rank-1 matmul; b2 via host-side rank-1
  s1 = S@1 correction.
"""
import numpy as np

N, E, NNZ = 50000, 20000, 1600000
CONCAT, C_IN, C_HID = 4, 256, 256
C = CONCAT * C_HID            # 1024
C_OUT, C_OUT_P = 40, 64
W = 8
NPC_R, EPC_R = N // W, E // W           # 6250, 2500 real per core
NBLK, EBLK = 49, 20
NPC, EPC = NBLK * 128, EBLK * 128       # 6272, 2560 padded per core
NP_, EP_ = W * NPC, W * EPC             # 50176, 20480
NBLK_F, EBLK_F = NP_ // 128, EP_ // 128  # 392, 160 full blocks
NHALF = NP_ // 2                        # 25088
BATCH = 8                                # gather chunks per dma_gather
E_SPLIT = (28, 21)                       # phase-E node-block passes (PSUM cap)


def _bf16(a):
    import ml_dtypes
    return np.asarray(a).astype(ml_dtypes.bfloat16)


def _wrap_idx(idx):
    """[L] int -> [128, L/16] int16 wrapped layout, replicated across q7 cores."""
    L = len(idx)
    assert L % 16 == 0
    a = np.full((16, L // 16), 0, np.int16)
    a[np.arange(L) % 16, np.arange(L) // 16] = idx.astype(np.int16)
    return np.tile(a, (8, 1))


def _streams(rows, segpos, nblk, K):
    """Build flat index stream [nblk*K*128] + seg table [128, nblk*K].
    rows/segpos: list per block of (row_ids, positions 0..127)."""
    L = nblk * K * 128
    idx = np.zeros(L, np.int64)
    seg = np.full((128, nblk * K), -1.0, np.float32)
    for b in range(nblk):
        r, p = rows[b], segpos[b]
        n = len(r)
        assert n <= K * 128
        base = b * K * 128
        idx[base:base + n] = r
        cols = b * K + np.arange(n) // 128
        seg[np.arange(n) % 128, cols] = p.astype(np.float32)
    return idx, seg


def _prep(node_idx, edge_idx, dv_is, de_inv):
    """All host-side index prep. Returns per-core dicts of arrays."""
    nrow = (node_idx // NPC_R) * NPC + node_idx % NPC_R    # node -> y row
    erow = (edge_idx // EPC_R) * EPC + edge_idx % EPC_R    # edge -> ef row
    # dir1: sort by edge
    p1 = np.argsort(edge_idx, kind="stable")
    e1, n1 = edge_idx[p1], nrow[p1]
    # dir2: sort by node
    p2 = np.argsort(node_idx, kind="stable")
    n2, e2 = node_idx[p2], erow[p2]
    per = []
    for c in range(W):
        m1 = (e1 >= c * EPC_R) & (e1 < (c + 1) * EPC_R)
        el = e1[m1] - c * EPC_R
        nr = n1[m1]
        lo_rows, lo_pos, hi_rows, hi_pos = [], [], [], []
        for b in range(EBLK):
            mb = (el >= b * 128) & (el < (b + 1) * 128)
            rb, pb = nr[mb], el[mb] - b * 128
            lo = rb < NHALF
            lo_rows.append(rb[lo]); lo_pos.append(pb[lo])
            hi_rows.append(rb[~lo] - NHALF); hi_pos.append(pb[~lo])
        m2 = (n2 >= c * NPC_R) & (n2 < (c + 1) * NPC_R)
        nl = n2[m2] - c * NPC_R
        er = e2[m2]
        c_rows, c_pos = [], []
        for b in range(NBLK):
            mb = (nl >= b * 128) & (nl < (b + 1) * 128)
            c_rows.append(er[mb]); c_pos.append(nl[mb] - b * 128)
        per.append((lo_rows, lo_pos, hi_rows, hi_pos, c_rows, c_pos))
    KA = max(max((len(r) + 127) // 128 for r in p[0]) for p in per)
    KB = max(max((len(r) + 127) // 128 for r in p[2]) for p in per)
    KC = max(max((len(r) + 127) // 128 for r in p[4]) for p in per)
    KA, KB, KC = max(KA, 1), max(KB, 1), max(KC, 1)
    cores = []
    for c in range(W):
        lo_rows, lo_pos, hi_rows, hi_pos, c_rows, c_pos = per[c]
        iA, sA = _streams(lo_rows, lo_pos, EBLK, KA)
        iB, sB = _streams(hi_rows, hi_pos, EBLK, KB)
        iC, sC = _streams(c_rows, c_pos, NBLK, KC)
        dv = np.zeros(NPC, np.float32)
        dv[:NPC_R] = dv_is[c * NPC_R:(c + 1) * NPC_R]
        de = np.zeros(EPC, np.float32)
        de[:EPC_R] = de_inv[c * EPC_R:(c + 1) * EPC_R]
        cores.append(dict(
            idxA=_wrap_idx(iA), segA=_bf16(sA), idxB=_wrap_idx(iB), segB=_bf16(sB),
            idxC=_wrap_idx(iC), segC=_bf16(sC),
            dv_blk=dv.reshape(NBLK, 128).T.copy(),
            de_blk=de.reshape(EBLK, 128).T.copy()))
    return cores, KA, KB, KC


def _prep_dense(node_idx, edge_idx, dv_is, de_inv):
    """Host-baked dense P = Dv^-1/2 H De^-1/2 block slices for the 2nd smooth.
    Pd[c]: [NBLK_F, 128, EPC]  (all node blocks x core c's edge cols)
    Pe[c]: [EBLK_F, 128, NPC]  (all edge blocks x core c's node cols)"""
    nrow = ((node_idx // NPC_R) * NPC + node_idx % NPC_R).astype(np.int64)
    erow = ((edge_idx // EPC_R) * EPC + edge_idx % EPC_R).astype(np.int64)
    sq_de = np.sqrt(de_inv)
    val = (dv_is[node_idx] * sq_de[edge_idx]).astype(np.float32)
    ec = edge_idx // EPC_R
    ncore = node_idx // NPC_R
    Pd, Pe = [], []
    for c in range(W):
        m = ec == c
        el = (edge_idx[m] - c * EPC_R).astype(np.int64)
        flat = np.zeros(NP_ * EPC, np.float32)
        np.add.at(flat, nrow[m] * EPC + el, val[m])
        Pd.append(_bf16(flat).reshape(NBLK_F, 128, EPC))
        del flat
        m = ncore == c
        nl = (nrow[m] - c * NPC).astype(np.int64)
        flat = np.zeros(EP_ * NPC, np.float32)
        np.add.at(flat, erow[m] * NPC + nl, val[m])
        Pe.append(_bf16(flat).reshape(EBLK_F, 128, NPC))
        del flat
    return Pd, Pe


def _build(KA, KB, KC):
    import concourse.bass as bass
    import concourse.mybir as mybir
    from concourse import bacc, masks
    from concourse.tile import TileContext

    f32, bf16, i16 = mybir.dt.float32, mybir.dt.bfloat16, mybir.dt.int16
    nc = bacc.Bacc("TRN2", num_devices=W)
    T = lambda n, s, d=f32: nc.dram_tensor(n, s, d, kind="ExternalInput")
    xT = T("xT", [CONCAT, C_IN, NPC], bf16)
    W1 = T("W1", [CONCAT, C_IN, C_HID], bf16)
    b1c = T("b1c", [1, C], bf16)
    W2p = T("W2p", [C, C_OUT_P], bf16)
    Pd = T("Pd", [NBLK_F, 128, EPC], bf16)
    Pe = T("Pe", [EBLK_F, 128, NPC], bf16)
    dv_blk = T("dv_blk", [128, NBLK])
    de_blk = T("de_blk", [128, EBLK])
    idxA = T("idxA", [128, EBLK * KA * 8], i16); segA = T("segA", [128, EBLK * KA], bf16)
    idxB = T("idxB", [128, EBLK * KB * 8], i16); segB = T("segB", [128, EBLK * KB], bf16)
    idxC = T("idxC", [128, NBLK * KC * 8], i16); segC = T("segC", [128, NBLK * KC], bf16)
    iota_d = T("iota", [128, 128], bf16)
    outT_own = nc.dram_tensor("outT_own", [C_OUT_P, NPC], f32, kind="ExternalOutput")
    I = lambda n, s, d: nc.dram_tensor(n, s, d, kind="Internal")
    S = lambda n, s, d: nc.dram_tensor(n, s, d, kind="Internal", addr_space="Shared")
    y_own, y_full = I("y_own", [NPC, C], bf16), S("y_full", [NP_, C], bf16)
    ef_own, ef_full = I("ef_own", [EPC, C], bf16), S("ef_full", [EP_, C], bf16)
    y2_own, y2_full = I("y2_own", [NPC, C_OUT_P], bf16), S("y2_full", [NP_, C_OUT_P], bf16)
    z_own, z_full = I("z_own", [EPC, C_OUT_P], bf16), S("z_full", [EP_, C_OUT_P], bf16)
    RG = [list(range(W))]
    AG = lambda i, o: nc.gpsimd.collective_compute(
        "AllGather", mybir.AluOpType.bypass, replica_groups=RG, ins=[i[:]], outs=[o[:]])

    with TileContext(nc) as tc:
        with tc.tile_pool(name="const", bufs=1) as cp:
            w1_sb = cp.tile([128, CONCAT * 2 * C_HID], bf16)      # [k][q] -> 256 cols each
            for k in range(CONCAT):
                for q in range(2):
                    nc.sync.dma_start(
                        w1_sb[:, (k * 2 + q) * C_HID:(k * 2 + q + 1) * C_HID],
                        W1[k, q * 128:(q + 1) * 128, :])
            w2_sb = cp.tile([128, 8 * C_OUT_P], bf16)
            for f in range(8):
                nc.sync.dma_start(w2_sb[:, f * C_OUT_P:(f + 1) * C_OUT_P],
                                  W2p[f * 128:(f + 1) * 128, :])
            b1_sb = cp.tile([1, C], bf16); nc.sync.dma_start(b1_sb[:], b1c[:])
            ones_sb = cp.tile([1, 128], bf16); nc.vector.memset(ones_sb[:], 1.0)
            iota_sb = cp.tile([128, 128], bf16); nc.sync.dma_start(iota_sb[:], iota_d[:])
            ident = cp.tile([128, 128], bf16); masks.make_identity(nc, ident[:])
            dv_sb = cp.tile([128, NBLK], f32); nc.sync.dma_start(dv_sb[:], dv_blk[:])
            de_sb = cp.tile([128, EBLK], f32); nc.sync.dma_start(de_sb[:], de_blk[:])
            iA = cp.tile([128, EBLK * KA * 8], i16); nc.sync.dma_start(iA[:], idxA[:])
            iB = cp.tile([128, EBLK * KB * 8], i16); nc.sync.dma_start(iB[:], idxB[:])
            iC = cp.tile([128, NBLK * KC * 8], i16); nc.sync.dma_start(iC[:], idxC[:])
            sA = cp.tile([128, EBLK * KA], bf16); nc.sync.dma_start(sA[:], segA[:])
            sB = cp.tile([128, EBLK * KB], bf16); nc.sync.dma_start(sB[:], segB[:])
            sC = cp.tile([128, NBLK * KC], bf16); nc.sync.dma_start(sC[:], segC[:])

            mm = lambda *a, **kw: nc.tensor.matmul(*a, skip_group_check=True, **kw)

            def seg_pass(K, idx_sb, seg_sb, src_ap, elem, pool, psum_ap,
                         start_stream, stop_stream):
                """Gather+one-hot-matmul accumulation over one block's stream."""
                nbat = (K + BATCH - 1) // BATCH
                for s in range(nbat):
                    k0 = s * BATCH
                    nch = min(BATCH, K - s * BATCH)
                    g = pool.tile([128, BATCH, elem], bf16, tag="gat")
                    nc.gpsimd.dma_gather(
                        out_ap=g[:, :nch, :], in_ap=src_ap,
                        idxs_ap=idx_sb[:, k0 * 8:(k0 + nch) * 8],
                        num_idxs=nch * 128, num_idxs_reg=nch * 128,
                        elem_size=elem)
                    oh = pool.tile([128, BATCH, 128], bf16, tag="oh")
                    nc.vector.tensor_tensor(
                        out=oh[:, :nch, :],
                        in0=iota_sb[:, None, :].broadcast_to([128, nch, 128]),
                        in1=seg_sb[:, k0:k0 + nch, None].broadcast_to([128, nch, 128]),
                        op=mybir.AluOpType.is_equal)
                    for j in range(nch):
                        first = start_stream and (s == 0 and j == 0)
                        last = stop_stream and (k0 + j == K - 1)
                        for h in range((elem + 511) // 512):
                            w_ = min(512, elem - h * 512)
                            mm(psum_ap[:, h * 512:h * 512 + w_],
                               lhsT=oh[:, j, :], rhs=g[:, j, h * 512:h * 512 + w_],
                               start=first, stop=last)

            # ---- phase A: y = dv * (x @ W1 + 1 b1) ----
            with nc.named_scope("phA"), \
                 tc.tile_pool(name="pa", bufs=3) as pa, \
                 tc.tile_pool(name="pap", bufs=2, space="PSUM") as pap:
                for b in range(NBLK):
                    ps = pap.tile([128, C], f32, tag="psA")
                    mm(ps[:, :512], lhsT=ones_sb[:, :], rhs=b1_sb[:, :512], start=True, stop=False)
                    mm(ps[:, 512:], lhsT=ones_sb[:, :], rhs=b1_sb[:, 512:], start=True, stop=False)
                    for k in range(CONCAT):
                        for q in range(2):
                            xt = pa.tile([128, 128], bf16, tag="xt")
                            nc.sync.dma_start(xt[:], xT[k, q * 128:(q + 1) * 128,
                                                        b * 128:(b + 1) * 128])
                            mm(ps[:, k * C_HID:(k + 1) * C_HID], lhsT=xt[:],
                               rhs=w1_sb[:, (k * 2 + q) * C_HID:(k * 2 + q + 1) * C_HID],
                               start=False, stop=(q == 1))
                    y_sb = pa.tile([128, C], bf16, tag="ysb")
                    nc.vector.tensor_tensor(
                        out=y_sb[:], in0=ps[:],
                        in1=dv_sb[:, b:b + 1].broadcast_to([128, C]),
                        op=mybir.AluOpType.mult)
                    nc.sync.dma_start(y_own[b * 128:(b + 1) * 128, :], y_sb[:])
            with nc.named_scope("AGy"):
                AG(y_own, y_full)

            # ---- phase B: ef = de * (H^T y) over own edges ----
            with nc.named_scope("phB"), \
                 tc.tile_pool(name="pb", bufs=3) as pb, \
                 tc.tile_pool(name="pbp", bufs=2, space="PSUM") as pbp:
                for b in range(EBLK):
                    psB = pbp.tile([128, C], f32, tag="psB")
                    seg_pass(KA, iA[:, b * KA * 8:], sA[:, b * KA:], y_full[0:NHALF, :],
                             C, pb, psB, True, False)
                    seg_pass(KB, iB[:, b * KB * 8:], sB[:, b * KB:],
                             y_full[NHALF:NP_, :], C, pb, psB, False, True)
                    ef_sb = pb.tile([128, C], bf16, tag="efsb")
                    nc.vector.tensor_tensor(
                        out=ef_sb[:], in0=psB[:],
                        in1=de_sb[:, b:b + 1].broadcast_to([128, C]),
                        op=mybir.AluOpType.mult)
                    nc.sync.dma_start(ef_own[b * 128:(b + 1) * 128, :], ef_sb[:])
            with nc.named_scope("AGef"):
                AG(ef_own, ef_full)

            # ---- phase C: u = relu(H ef); y2 = dv * (u @ W2) ----
            with nc.named_scope("phC"), \
                 tc.tile_pool(name="pc", bufs=3) as pc, \
                 tc.tile_pool(name="pcp", bufs=2, space="PSUM") as pcp, \
                 tc.tile_pool(name="pct", bufs=1, space="PSUM") as pct:
                for b in range(NBLK):
                    pz = pcp.tile([128, C], f32, tag="psC")
                    seg_pass(KC, iC[:, b * KC * 8:], sC[:, b * KC:], ef_full[:],
                             C, pc, pz, True, True)
                    u_sb = pc.tile([128, C], bf16, tag="usb")
                    nc.scalar.activation(out=u_sb[:], in_=pz[:],
                                         func=mybir.ActivationFunctionType.Relu)
                    pt = pct.tile([128, C], bf16, tag="ptC")
                    for f in range(8):
                        nc.tensor.transpose(pt[:, f * 128:(f + 1) * 128],
                                            u_sb[:, f * 128:(f + 1) * 128], ident[:])
                    ut_sb = pc.tile([128, C], bf16, tag="utsb")
                    nc.vector.tensor_copy(ut_sb[:], pt[:])
                    po = pct.tile([128, C_OUT_P], f32, tag="poC")
                    for f in range(8):
                        mm(po[:], lhsT=ut_sb[:, f * 128:(f + 1) * 128],
                           rhs=w2_sb[:, f * C_OUT_P:(f + 1) * C_OUT_P],
                           start=(f == 0), stop=(f == 7))
                    y2_sb = pc.tile([128, C_OUT_P], bf16, tag="y2sb")
                    nc.vector.tensor_tensor(
                        out=y2_sb[:], in0=po[:],
                        in1=dv_sb[:, b:b + 1].broadcast_to([128, C_OUT_P]),
                        op=mybir.AluOpType.mult)
                    nc.sync.dma_start(y2_own[b * 128:(b + 1) * 128, :], y2_sb[:])
            with nc.named_scope("AGy2"):
                AG(y2_own, y2_full)

            # ---- phase D: Z^T = (P^T y2)^T over own edge cols (dense stream) ----
            with nc.named_scope("phD"), \
                 tc.tile_pool(name="pdd", bufs=3) as pdd:
                with tc.tile_pool(name="pdp", bufs=1, space="PSUM") as pdp:
                    zt_ps = pdp.tile([C_OUT_P, EPC], f32, tag="ztps")
                    for nb in range(NBLK_F):
                        y2t = pdd.tile([128, C_OUT_P], bf16, tag="y2t")
                        nc.sync.dma_start(y2t[:], y2_full[nb * 128:(nb + 1) * 128, :])
                        pblk = pdd.tile([128, EPC], bf16, tag="pblk")
                        nc.sync.dma_start(pblk[:], Pd[nb, :, :])
                        for h in range(EPC // 512):
                            mm(zt_ps[:, h * 512:(h + 1) * 512], lhsT=y2t[:],
                               rhs=pblk[:, h * 512:(h + 1) * 512],
                               start=(nb == 0), stop=(nb == NBLK_F - 1))
                    zt_sb = pdd.tile([128, EPC], bf16, tag="ztsb")
                    nc.vector.memset(zt_sb[C_OUT_P:128, :], 0.0)
                    nc.vector.tensor_copy(zt_sb[:C_OUT_P, :], zt_ps[:])
                with tc.tile_pool(name="pdp2", bufs=2, space="PSUM") as pdp2:
                    for eb in range(EBLK):
                        ptz = pdp2.tile([128, 128], bf16, tag="ptz")
                        nc.tensor.transpose(ptz[:], zt_sb[:, eb * 128:(eb + 1) * 128],
                                            ident[:])
                        z_sb = pdd.tile([128, C_OUT_P], bf16, tag="zsb")
                        nc.vector.tensor_copy(z_sb[:], ptz[:, :C_OUT_P])
                        nc.sync.dma_start(z_own[eb * 128:(eb + 1) * 128, :], z_sb[:])
            with nc.named_scope("AGz"):
                AG(z_own, z_full)

            # ---- phase E: res^T = (P Z)^T over own node cols (dense stream) ----
            with nc.named_scope("phE"), \
                 tc.tile_pool(name="pee", bufs=3) as pee, \
                 tc.tile_pool(name="pep", bufs=1, space="PSUM") as pep:
                nb0 = 0
                for npass, nblks in enumerate(E_SPLIT):
                    ncols = nblks * 128
                    rT = pep.tile([C_OUT_P, ncols], f32, tag="rT")
                    for eb in range(EBLK_F):
                        z_t = pee.tile([128, C_OUT_P], bf16, tag="zt")
                        nc.sync.dma_start(z_t[:], z_full[eb * 128:(eb + 1) * 128, :])
                        pe_t = pee.tile([128, ncols], bf16, tag="pet%d" % npass)
                        nc.sync.dma_start(pe_t[:],
                                          Pe[eb, :, nb0 * 128:nb0 * 128 + ncols])
                        for h0 in range(0, ncols, 512):
                            w_ = min(512, ncols - h0)
                            mm(rT[:, h0:h0 + w_], lhsT=z_t[:], rhs=pe_t[:, h0:h0 + w_],
                               start=(eb == 0), stop=(eb == EBLK_F - 1))
                    ot = pee.tile([C_OUT_P, ncols], f32, tag="ot%d" % npass)
                    nc.vector.tensor_copy(ot[:], rT[:])
                    nc.sync.dma_start(outT_own[:, nb0 * 128:nb0 * 128 + ncols], ot[:])
                    nb0 += nblks
    nc.finalize()
    return nc


_CACHE = {}


def kernel(x_list, W1, b1, W2, b2, node_idx, edge_idx, n_edges, _trace=False):
    from concourse import bass_utils
    x_list = np.asarray(x_list, np.float32); W1 = np.asarray(W1, np.float32)
    b1 = np.asarray(b1, np.float32); W2 = np.asarray(W2, np.float32)
    b2 = np.asarray(b2, np.float32)
    node_idx = np.asarray(node_idx, np.int32); edge_idx = np.asarray(edge_idx, np.int32)

    dv = np.bincount(node_idx, minlength=N).astype(np.float32)
    de = np.bincount(edge_idx, minlength=E).astype(np.float32)
    dv_is = np.where(dv > 0, 1.0 / np.sqrt(np.maximum(dv, 1.0)), 0.0).astype(np.float32)
    de_inv = np.where(de > 0, 1.0 / np.maximum(de, 1.0), 0.0).astype(np.float32)
    # s1 = S @ 1 for the host-side b2 rank-1 term
    ef_t = np.bincount(edge_idx, weights=dv_is[node_idx], minlength=E) * de_inv
    s1 = dv_is * np.bincount(node_idx, weights=ef_t[edge_idx], minlength=N)

    cores, KA, KB, KC = _prep(node_idx, edge_idx, dv_is, de_inv)
    Pd, Pe = _prep_dense(node_idx, edge_idx, dv_is, de_inv)
    key = (KA, KB, KC)
    if key not in _CACHE:
        _CACHE[key] = _build(KA, KB, KC)
    nc = _CACHE[key]

    W2p = np.zeros((C, C_OUT_P), np.float32)
    W2p[:, :C_OUT] = W2
    iota_np = np.tile(np.arange(128, dtype=np.float32), (128, 1))
    in_maps = []
    for c in range(W):
        xTc = np.zeros((CONCAT, C_IN, NPC), np.float32)
        xTc[:, :, :NPC_R] = x_list[:, c * NPC_R:(c + 1) * NPC_R, :].transpose(0, 2, 1)
        m = dict(xT=_bf16(xTc), W1=_bf16(W1), b1c=_bf16(b1.reshape(1, C)),
                 W2p=_bf16(W2p), iota=_bf16(iota_np), Pd=Pd[c], Pe=Pe[c],
                 **cores[c])
        in_maps.append(m)
    try:
        res = bass_utils.run_bass_kernel_spmd(nc, in_maps, core_ids=list(range(W)),
                                              trace=_trace)
    except ModuleNotFoundError:
        res = bass_utils.run_bass_kernel_spmd(nc, in_maps, core_ids=list(range(W)),
                                              trace=False)
    out = np.empty((N, C_OUT), np.float32)
    for c in range(W):
        out[c * NPC_R:(c + 1) * NPC_R] = res.results[c]["outT_own"][:C_OUT, :NPC_R].T
    out += np.outer(s1, b2)
    kernel._last = res
    return out
